# revision 3
# baseline (speedup 1.0000x reference)
"""Ragged class-token prepend (packed layout) on 8 Trainium2 NeuronCores.

Op: given x_flat [T, D] (packed rows of B ragged sequences, seg_ids sorted),
produce [T+B, D] where each sequence gains one leading class-token row
(the [1, D] weight).

Strategy (data-parallel over output rows, per the sharding hint):
  - Output rows are split evenly across 8 cores (R = (T+B)/8 rows each).
  - Each core receives a contiguous R-row window of x_flat (+ the weight
    appended as row R) and small int32 index tensors.
  - Main pass: because seg_ids are sorted, output rows are long runs of
    consecutive input rows broken only at the B class-token insertions.
    43 indirect-DMA block gathers (K=6 consecutive rows per 6KB descriptor,
    near-sequential addresses) pipelined against 43 big contiguous HWDGE
    grid writes.  This pass alone runs at the HBM bus floor: the SWDGE
    gathers issue far ahead of the data so reads and writes stream
    back-to-back.
  - Fixup pass (~3% of rows whose K-run crosses a class-token insertion):
    per-row gather+scatter with three stall-avoidance measures over the
    naive form: (1) fixup gathers issue right after the main gathers (they
    only read x_in, so they hide inside the data drain), (2) each fixup
    scatter depends only on the grid writes whose row range it intersects
    (not a barrier on all 43), (3) padding slots use out-of-bounds indices
    that bounds_check + oob_is_err=False makes the DMA engine skip with
    zero traffic.
All heavy data movement happens on device; the host only computes index
arrays and slices inputs.
"""

import numpy as np

import concourse.bass as bass
import concourse.bacc as bacc
import concourse.mybir as mybir
from concourse.tile import TileContext, add_dep_helper
from concourse.bass_utils import run_bass_kernel_spmd

NCORES = 8
P = 128          # SBUF partitions
K = 6            # consecutive rows per gather descriptor

_program_cache: dict = {}


def build_program(R: int, D: int, k: int, nf: int, dep_ranges: tuple,
                  repeat: int = 1, bufs: int = 8):
    """SPMD program for one core.

    x_in:    [R+1, D] f32 (row R is the class-token weight row)
    blk_idx: [128, nblk] int32 - descriptor start row per (partition, block)
    fix_src/fix_dst: [128, nf] int32 - per-row fix entries (OOB-padded)
    out:     [R, D] f32
    dep_ranges[f] = (lo_block, hi_block) inclusive: the grid writes fixup
    scatter op f must wait for (union over cores).
    repeat: run the body N times (hardware loop) - for benchmarking only.
    """
    rows_pp = R // P
    blocks = [k] * (rows_pp // k)
    if rows_pp % k:
        blocks.append(rows_pp % k)
    nblk = len(blocks)
    # Bacc (not raw Bass): its compile() pass legalizes multi-sem waits
    nc = bacc.Bacc(num_devices=1)
    x_in = nc.dram_tensor("x_in", [R + 1, D], mybir.dt.float32, kind="ExternalInput")
    blk_idx = nc.dram_tensor("blk_idx", [P, nblk], mybir.dt.int32,
                             kind="ExternalInput")
    fix_src = nc.dram_tensor("fix_src", [P, nf], mybir.dt.int32,
                             kind="ExternalInput")
    fix_dst = nc.dram_tensor("fix_dst", [P, nf], mybir.dt.int32,
                             kind="ExternalInput")
    out = nc.dram_tensor("out", [R, D], mybir.dt.float32, kind="ExternalOutput")

    with TileContext(nc) as tc:
        with (
            tc.tile_pool(name="idxp", bufs=1) as idxp,
            tc.tile_pool(name="wp", bufs=bufs) as wp,
            tc.tile_pool(name="fp", bufs=nf + 1) as fp,
        ):
            bt = idxp.tile([P, nblk], mybir.dt.int32, tag="bt")
            fs = idxp.tile([P, nf], mybir.dt.int32, tag="fs")
            fd = idxp.tile([P, nf], mybir.dt.int32, tag="fd")
            nc.sync.dma_start(bt[:], blk_idx[:])
            nc.sync.dma_start(fs[:], fix_src[:])
            nc.sync.dma_start(fd[:], fix_dst[:])

            def body():
                # main pass: block gathers + contiguous grid writes
                writes = []
                off = 0
                for b, kb in enumerate(blocks):
                    wt = wp.tile([P, k * D], mybir.dt.float32, tag="wt")
                    nc.gpsimd.indirect_dma_start(
                        out=wt[:, : kb * D], out_offset=None, in_=x_in[:],
                        in_offset=bass.IndirectOffsetOnAxis(
                            ap=bt[:, b : b + 1], axis=0))
                    w = nc.sync.dma_start(
                        out[off : off + P * kb, :].rearrange(
                            "(p k) c -> p (k c)", p=P),
                        wt[:, : kb * D])
                    writes.append(w)
                    off += P * kb

                # fixup gathers: no deps, they hide inside the data drain
                ftiles = []
                for f in range(nf):
                    ft = fp.tile([P, D], mybir.dt.float32, tag=f"ft{f}")
                    nc.gpsimd.indirect_dma_start(
                        out=ft[:], out_offset=None, in_=x_in[:],
                        in_offset=bass.IndirectOffsetOnAxis(
                            ap=fs[:, f : f + 1], axis=0),
                        bounds_check=R, oob_is_err=False)
                    ftiles.append(ft)

                # fixup scatters: dep only on the LAST intersecting grid
                # write - HWDGE writes on one ring complete in FIFO order,
                # so write hi done implies writes < hi done
                for f in range(nf):
                    sc = nc.gpsimd.indirect_dma_start(
                        out=out[:],
                        out_offset=bass.IndirectOffsetOnAxis(
                            ap=fd[:, f : f + 1], axis=0),
                        in_=ftiles[f][:], in_offset=None,
                        bounds_check=R - 1, oob_is_err=False)
                    lo, hi = dep_ranges[f]
                    if hi >= lo:
                        b = min(hi, len(writes) - 1)
                        add_dep_helper(sc.ins, writes[b].ins,
                                       reason="fixup after its grid writes")

            if repeat == 1:
                body()
            else:
                with tc.For_i(0, repeat, 1):
                    body()
    nc.compile()
    return nc


def shard_inputs(x_flat, weight, seg_ids, num_segments, k: int = K):
    """Host-side index computation + slicing.

    Returns (in_maps, R, D, nf, dep_ranges)."""
    x_flat = np.asarray(x_flat)
    weight = np.asarray(weight, dtype=x_flat.dtype).reshape(1, -1)
    seg_ids = np.asarray(seg_ids)
    T, D = x_flat.shape
    B = int(num_segments)
    N = T + B
    assert N % (NCORES * P) == 0, (T, B)
    R = N // NCORES
    rows_pp = R // P
    blocks = [k] * (rows_pp // k)
    if rows_pp % k:
        blocks.append(rows_pp % k)

    # source row (into x_flat) for every output row; -1 marks class rows
    offsets = np.searchsorted(seg_ids, np.arange(B, dtype=seg_ids.dtype))
    src = np.empty(N, dtype=np.int64)
    src[offsets + np.arange(B)] = -1
    src[np.arange(T) + seg_ids + 1] = np.arange(T)

    # per-row position within its descriptor for the block layout
    pos_l = []
    for kb in blocks:
        jj = np.arange(P * kb)
        pos_l.append(jj % kb)
    pos = np.concatenate(pos_l)

    cores = []
    max_fix = 1
    for c in range(NCORES):
        s = src[c * R : (c + 1) * R]
        tok = s >= 0
        if tok.any():
            # token sources within a core are a consecutive ascending range
            w0 = int(s[np.argmax(tok)])
            w0 = max(0, min(w0, T - R))
        else:
            w0 = 0
        lidx = np.where(tok, s - w0, R).astype(np.int64)  # class rows -> R

        # descriptor start rows + expected block-pass value per row
        start_rows = np.empty(R, np.int64)
        off = 0
        for b, kb in enumerate(blocks):
            st = np.minimum(lidx[off + np.arange(P) * kb], R + 1 - kb)
            start_rows[off : off + P * kb] = np.repeat(st, kb)
            off += P * kb
        expected = start_rows + pos
        fix = np.nonzero(expected != lidx)[0]
        cores.append((w0, lidx, start_rows, fix))
        max_fix = max(max_fix, len(fix))

    nf = -(-max_fix // P)

    # per-fixup-op write-dep block ranges, union over cores
    blk_of = np.empty(R, np.int64)
    off = 0
    for b, kb in enumerate(blocks):
        blk_of[off : off + P * kb] = b
        off += P * kb
    dep_lo = np.full(nf, len(blocks), np.int64)
    dep_hi = np.full(nf, -1, np.int64)
    for c in range(NCORES):
        fix = cores[c][3]
        for f in range(nf):
            rows = fix[f * P : (f + 1) * P]
            if len(rows):
                dep_lo[f] = min(dep_lo[f], blk_of[rows.min()])
                dep_hi[f] = max(dep_hi[f], blk_of[rows.max()])
    dep_ranges = tuple(
        (int(lo), int(hi)) if hi >= lo else (0, -1)
        for lo, hi in zip(dep_lo, dep_hi))

    in_maps = []
    for c in range(NCORES):
        w0, lidx, start_rows, fix = cores[c]
        x_in = np.concatenate([x_flat[w0 : w0 + R], weight], axis=0)
        nblk = len(blocks)
        stm = np.empty((nblk, P), np.int64)
        off = 0
        for b, kb in enumerate(blocks):
            stm[b] = start_rows[off : off + P * kb : kb]
            off += P * kb
        blk_idx = np.ascontiguousarray(stm.T).astype(np.int32)

        # pad with OOB indices: bounds_check makes the DMA skip them
        pad = nf * P - len(fix)
        fdst = np.concatenate([fix, np.full(pad, R, np.int64)])
        fsrc = np.concatenate([lidx[fix], np.full(pad, R + 1, np.int64)])
        fdst2 = np.ascontiguousarray(fdst.reshape(nf, P).T).astype(np.int32)
        fsrc2 = np.ascontiguousarray(fsrc.reshape(nf, P).T).astype(np.int32)
        in_maps.append({"x_in": x_in, "blk_idx": blk_idx,
                        "fix_src": fsrc2, "fix_dst": fdst2})
    return in_maps, R, D, nf, dep_ranges


def kernel_run(inputs: dict, trace: bool = False, repeat: int = 1,
               k: int = K, bufs: int = 8, **spmd_kwargs):
    """Run the full op; returns (output, BassKernelResults)."""
    in_maps, R, D, nf, dep_ranges = shard_inputs(**inputs, k=k)
    key = (R, D, k, nf, dep_ranges, repeat, bufs)
    if key not in _program_cache:
        _program_cache[key] = build_program(
            R, D, k, nf, dep_ranges, repeat=repeat, bufs=bufs)
    nc = _program_cache[key]
    res = run_bass_kernel_spmd(
        nc, in_maps, list(range(NCORES)), trace=trace, **spmd_kwargs)
    out = np.concatenate([res.results[i]["out"] for i in range(NCORES)], axis=0)
    return out, res


def kernel(**inputs) -> np.ndarray:
    out, _ = kernel_run(inputs)
    return out


# revision 4
# speedup vs baseline: 1.0764x; 1.0764x over previous
"""Ragged class-token prepend (packed layout) on 8 Trainium2 NeuronCores.

Op: given x_flat [T, D] (packed rows of B ragged sequences, seg_ids sorted),
produce [T+B, D] where each sequence gains one leading class-token row
(the [1, D] weight).

Strategy (data-parallel over output rows, per the sharding hint):
  - Output rows are split evenly across 8 cores (R = (T+B)/8 rows each).
  - Each core receives a contiguous R-row window of x_flat (+ the weight
    appended as row R) and small int32 index tensors.
  - Main pass: because seg_ids are sorted, output rows are long runs of
    consecutive input rows broken only at the B class-token insertions.
    43 indirect-DMA block gathers (K=6 consecutive rows per 6KB descriptor,
    near-sequential addresses) pipelined against 43 big contiguous HWDGE
    grid writes.  This pass alone runs at the HBM bus floor: the SWDGE
    gathers issue far ahead of the data so reads and writes stream
    back-to-back.
  - Fixup pass (~3% of rows whose K-run crosses a class-token insertion):
    per-row gather+scatter with three stall-avoidance measures over the
    naive form: (1) fixup gathers issue right after the main gathers (they
    only read x_in, so they hide inside the data drain), (2) each fixup
    scatter depends only on the grid writes whose row range it intersects
    (not a barrier on all 43), (3) padding slots use out-of-bounds indices
    that bounds_check + oob_is_err=False makes the DMA engine skip with
    zero traffic.
All heavy data movement happens on device; the host only computes index
arrays and slices inputs.
"""

import numpy as np

import concourse.bass as bass
import concourse.bacc as bacc
import concourse.mybir as mybir
from concourse.tile import TileContext, add_dep_helper
from concourse.bass_utils import run_bass_kernel_spmd

NCORES = 8
P = 128          # SBUF partitions
K = 6            # consecutive rows per gather descriptor

_program_cache: dict = {}


def build_program(R: int, D: int, k: int, nf: int, dep_ranges: tuple,
                  repeat: int = 1, bufs: int = 8):
    """SPMD program for one core.

    x_in:    [R+1, D] f32 (row R is the class-token weight row)
    blk_idx: [128, nblk] int32 - descriptor start row per (partition, block)
    fix_src/fix_dst: [128, nf] int32 - per-row fix entries (OOB-padded)
    out:     [R, D] f32
    dep_ranges[f] = (lo_block, hi_block) inclusive: the grid writes fixup
    scatter op f must wait for (union over cores).
    repeat: run the body N times (hardware loop) - for benchmarking only.
    """
    rows_pp = R // P
    blocks = [k] * (rows_pp // k)
    if rows_pp % k:
        blocks.append(rows_pp % k)
    nblk = len(blocks)
    # Bacc (not raw Bass): its compile() pass legalizes multi-sem waits
    nc = bacc.Bacc(num_devices=1)
    x_in = nc.dram_tensor("x_in", [R + 1, D], mybir.dt.float32, kind="ExternalInput")
    blk_idx = nc.dram_tensor("blk_idx", [P, nblk], mybir.dt.int32,
                             kind="ExternalInput")
    fix_src = nc.dram_tensor("fix_src", [P, nf], mybir.dt.int32,
                             kind="ExternalInput")
    fix_dst = nc.dram_tensor("fix_dst", [P, nf], mybir.dt.int32,
                             kind="ExternalInput")
    out = nc.dram_tensor("out", [R, D], mybir.dt.float32, kind="ExternalOutput")

    with TileContext(nc) as tc:
        with (
            tc.tile_pool(name="idxp", bufs=1) as idxp,
            tc.tile_pool(name="wp", bufs=bufs) as wp,
            tc.tile_pool(name="fp", bufs=nf + 1) as fp,
        ):
            bt = idxp.tile([P, nblk], mybir.dt.int32, tag="bt")
            fs = idxp.tile([P, nf], mybir.dt.int32, tag="fs")
            fd = idxp.tile([P, nf], mybir.dt.int32, tag="fd")
            nc.sync.dma_start(bt[:], blk_idx[:])
            nc.sync.dma_start(fs[:], fix_src[:])
            nc.sync.dma_start(fd[:], fix_dst[:])

            def body():
                # main pass: block gathers + contiguous grid writes
                writes = []
                off = 0
                for b, kb in enumerate(blocks):
                    wt = wp.tile([P, k * D], mybir.dt.float32, tag="wt")
                    nc.gpsimd.indirect_dma_start(
                        out=wt[:, : kb * D], out_offset=None, in_=x_in[:],
                        in_offset=bass.IndirectOffsetOnAxis(
                            ap=bt[:, b : b + 1], axis=0))
                    w = nc.sync.dma_start(
                        out[off : off + P * kb, :].rearrange(
                            "(p k) c -> p (k c)", p=P),
                        wt[:, : kb * D])
                    writes.append(w)
                    off += P * kb

                # fixup gathers: no deps, they hide inside the data drain
                ftiles = []
                for f in range(nf):
                    ft = fp.tile([P, D], mybir.dt.float32, tag=f"ft{f}")
                    nc.gpsimd.indirect_dma_start(
                        out=ft[:], out_offset=None, in_=x_in[:],
                        in_offset=bass.IndirectOffsetOnAxis(
                            ap=fs[:, f : f + 1], axis=0),
                        bounds_check=R, oob_is_err=False)
                    ftiles.append(ft)

                # fixup scatters: dep only on the LAST intersecting grid
                # write - HWDGE writes on one ring complete in FIFO order,
                # so write hi done implies writes < hi done
                for f in range(nf):
                    sc = nc.gpsimd.indirect_dma_start(
                        out=out[:],
                        out_offset=bass.IndirectOffsetOnAxis(
                            ap=fd[:, f : f + 1], axis=0),
                        in_=ftiles[f][:], in_offset=None,
                        bounds_check=R - 1, oob_is_err=False)
                    lo, hi = dep_ranges[f]
                    if hi >= lo:
                        b = min(hi, len(writes) - 1)
                        add_dep_helper(sc.ins, writes[b].ins,
                                       reason="fixup after its grid writes")

            if repeat == 1:
                body()
            else:
                with tc.For_i(0, repeat, 1):
                    body()
    nc.compile()
    return nc


def shard_inputs(x_flat, weight, seg_ids, num_segments, k: int = K):
    """Host-side index computation + slicing.

    Returns (in_maps, R, D, nf, dep_ranges)."""
    x_flat = np.asarray(x_flat)
    weight = np.asarray(weight, dtype=x_flat.dtype).reshape(1, -1)
    seg_ids = np.asarray(seg_ids)
    T, D = x_flat.shape
    B = int(num_segments)
    N = T + B
    assert N % (NCORES * P) == 0, (T, B)
    R = N // NCORES
    rows_pp = R // P
    blocks = [k] * (rows_pp // k)
    if rows_pp % k:
        blocks.append(rows_pp % k)

    # source row (into x_flat) for every output row; -1 marks class rows
    offsets = np.searchsorted(seg_ids, np.arange(B, dtype=seg_ids.dtype))
    src = np.empty(N, dtype=np.int64)
    src[offsets + np.arange(B)] = -1
    src[np.arange(T) + seg_ids + 1] = np.arange(T)

    # per-row position within its descriptor for the block layout
    pos_l = []
    for kb in blocks:
        jj = np.arange(P * kb)
        pos_l.append(jj % kb)
    pos = np.concatenate(pos_l)

    cores = []
    max_fix = 1
    for c in range(NCORES):
        s = src[c * R : (c + 1) * R]
        tok = s >= 0
        if tok.any():
            # token sources within a core are a consecutive ascending range
            w0 = int(s[np.argmax(tok)])
            w0 = max(0, min(w0, T - R))
        else:
            w0 = 0
        lidx = np.where(tok, s - w0, R).astype(np.int64)  # class rows -> R

        # descriptor start rows: for each K-row chunk pick the candidate
        # alignment (one per row j: start = lidx[j] - j) that leaves the
        # fewest broken rows - a class row at offset u then breaks only
        # min(u+1, K-u) rows instead of always K-u (first-row alignment)
        start_rows = np.empty(R, np.int64)
        off = 0
        for b, kb in enumerate(blocks):
            rows = lidx[off : off + P * kb].reshape(P, kb)
            cands = rows - np.arange(kb)                      # [P, kb]
            cnt = (
                rows[:, None, :]
                != cands[:, :, None] + np.arange(kb)[None, None, :]
            ).sum(axis=2)
            best = np.argmin(cnt, axis=1)
            st = np.clip(cands[np.arange(P), best], 0, R + 1 - kb)
            start_rows[off : off + P * kb] = np.repeat(st, kb)
            off += P * kb
        expected = start_rows + pos
        fix = np.nonzero(expected != lidx)[0]
        cores.append((w0, lidx, start_rows, fix))
        max_fix = max(max_fix, len(fix))

    nf = -(-max_fix // P)

    # per-fixup-op write-dep block ranges, union over cores
    blk_of = np.empty(R, np.int64)
    off = 0
    for b, kb in enumerate(blocks):
        blk_of[off : off + P * kb] = b
        off += P * kb
    dep_lo = np.full(nf, len(blocks), np.int64)
    dep_hi = np.full(nf, -1, np.int64)
    for c in range(NCORES):
        fix = cores[c][3]
        for f in range(nf):
            rows = fix[f * P : (f + 1) * P]
            if len(rows):
                dep_lo[f] = min(dep_lo[f], blk_of[rows.min()])
                dep_hi[f] = max(dep_hi[f], blk_of[rows.max()])
    dep_ranges = tuple(
        (int(lo), int(hi)) if hi >= lo else (0, -1)
        for lo, hi in zip(dep_lo, dep_hi))

    in_maps = []
    for c in range(NCORES):
        w0, lidx, start_rows, fix = cores[c]
        x_in = np.concatenate([x_flat[w0 : w0 + R], weight], axis=0)
        nblk = len(blocks)
        stm = np.empty((nblk, P), np.int64)
        off = 0
        for b, kb in enumerate(blocks):
            stm[b] = start_rows[off : off + P * kb : kb]
            off += P * kb
        blk_idx = np.ascontiguousarray(stm.T).astype(np.int32)

        # pad with OOB indices: bounds_check makes the DMA skip them
        pad = nf * P - len(fix)
        fdst = np.concatenate([fix, np.full(pad, R, np.int64)])
        fsrc = np.concatenate([lidx[fix], np.full(pad, R + 1, np.int64)])
        fdst2 = np.ascontiguousarray(fdst.reshape(nf, P).T).astype(np.int32)
        fsrc2 = np.ascontiguousarray(fsrc.reshape(nf, P).T).astype(np.int32)
        in_maps.append({"x_in": x_in, "blk_idx": blk_idx,
                        "fix_src": fsrc2, "fix_dst": fdst2})
    return in_maps, R, D, nf, dep_ranges


def kernel_run(inputs: dict, trace: bool = False, repeat: int = 1,
               k: int = K, bufs: int = 8, **spmd_kwargs):
    """Run the full op; returns (output, BassKernelResults)."""
    in_maps, R, D, nf, dep_ranges = shard_inputs(**inputs, k=k)
    key = (R, D, k, nf, dep_ranges, repeat, bufs)
    if key not in _program_cache:
        _program_cache[key] = build_program(
            R, D, k, nf, dep_ranges, repeat=repeat, bufs=bufs)
    nc = _program_cache[key]
    res = run_bass_kernel_spmd(
        nc, in_maps, list(range(NCORES)), trace=trace, **spmd_kwargs)
    out = np.concatenate([res.results[i]["out"] for i in range(NCORES)], axis=0)
    return out, res


def kernel(**inputs) -> np.ndarray:
    out, _ = kernel_run(inputs)
    return out
